# revision 14
# baseline (speedup 1.0000x reference)
"""Trainium2 Bass kernel for nn_Attention_43868795961547 (sparse_attention).

Reference computation per batch item (8 items, data-parallel over 8 cores):
  x  = LN(img[b]) @ w_qkv -> q,k,v (8 heads x 64)          [1024 tokens]
  kt,vt from LN(tab[b]) @ w_tab_qkv appended as key/value position 1024
  out = softmax(q k^T / 8) @ v ; out @ w_out + b_out        -> [1024, 512]

v2 strategy (per core), all matmuls bf16 (1-pass PE + FWL weight loads):
  - LN stats fp32 (bn_stats), xn cast bf16, PE-transposed; ln_w/ln_b applied
    as per-partition scale/bias in the PSUM->SBUF move (DVE).
  - qT,kT feature-major [512 x 1024] bf16; v token-major with per-head
    interleaved ones column (65-col groups) so attn@v emits softmax
    denominators for free.
  - tab token: k_t dots for all 8 heads in ONE packed M=8 matmul chain
    (zero-padded block lhsT), exp'd once; tab's rank-1 contribution to the
    output folded into the final projection as a K=8 accumulation row
    (lhsT = normalized tab weights, rhs = v_t @ w_out precomputed on PE).
  - dots^T[kpos, q] per head pair row-packed (K=64 tiles at rows 0/64);
    exp on ACT from PSUM (scale=1/8 folded), ae bf16.
  - attn@v: out^T[65, q] over 8 kpos chunks; denominators DMA-packed to
    rows 0..7, reciprocal_approx_fast, gpsimd partition_broadcast for the
    per-head normalize multiply.
  - PE issue order interleaves attn@v(hp) with dots(hp') so the PE rides
    the ACT exp pipeline without starving.
"""

import numpy as np

import concourse.bass as bass
import concourse.mybir as mybir
import concourse.tile as tile
from concourse import bacc
from concourse import bass_utils
from concourse.masks import make_identity

F32 = mybir.dt.float32
BF16 = mybir.dt.bfloat16

N_CORES = 8
NTOK = 1024  # img tokens per batch item
DIM = 512
HEADS = 8
DHEAD = 64
INNER = 512
SCALE = DHEAD ** -0.5  # 0.125
EPS = 1e-5

NT = NTOK // 128   # 8 token tiles
NC_ = DIM // 128   # 4 feature chunks
NQB = 2            # q blocks of 512
QB = 512


def build_program():
    nc = bacc.Bacc(
        "TRN2",
        target_bir_lowering=False,
        debug=False,
        enable_asserts=False,
        num_devices=N_CORES,
    )

    img = nc.dram_tensor("img_s", [NTOK, DIM], F32, kind="ExternalInput").ap()
    tab = nc.dram_tensor("tab_s", [1, DIM], F32, kind="ExternalInput").ap()
    w_qkv = nc.dram_tensor("wq_b", [DIM, 3 * INNER], BF16, kind="ExternalInput").ap()
    # only k,v columns of w_tab_qkv (cols 512:1536) are used
    w_tab = nc.dram_tensor("wt_b", [DIM, 2 * INNER], BF16, kind="ExternalInput").ap()
    w_out = nc.dram_tensor("wo_b", [INNER, DIM], BF16, kind="ExternalInput").ap()
    b_out = nc.dram_tensor("b_out", [1, DIM], F32, kind="ExternalInput").ap()
    ln_w = nc.dram_tensor("ln_w", [1, DIM], F32, kind="ExternalInput").ap()
    ln_b = nc.dram_tensor("ln_b", [1, DIM], F32, kind="ExternalInput").ap()
    out_d = nc.dram_tensor("out_s", [NTOK, DIM], F32, kind="ExternalOutput").ap()
    dscr = nc.dram_tensor("dscratch", [HEADS, NTOK], BF16, kind="Internal").ap()

    with tile.TileContext(nc) as tc:
        kernel_body(tc, img, tab, w_qkv, w_tab, w_out, b_out, ln_w, ln_b, out_d,
                    dscr)

    nc.compile()
    return nc


def kernel_body(tc, img, tab, w_qkv, w_tab, w_out, b_out, ln_w, ln_b, out_d,
                dscr):
    nc = tc.nc
    AF = mybir.ActivationFunctionType
    OP = mybir.AluOpType

    import contextlib
    ctx = contextlib.ExitStack()
    with ctx:
        # ---------------- pools ----------------
        const_p = ctx.enter_context(tc.tile_pool(name="const", bufs=1))
        qkT_p = ctx.enter_context(tc.tile_pool(name="qkT", bufs=1))
        v_p = ctx.enter_context(tc.tile_pool(name="vp", bufs=1))
        outuT_p = ctx.enter_context(tc.tile_pool(name="outuT", bufs=1))
        small_p = ctx.enter_context(tc.tile_pool(name="smallp", bufs=1))
        w_p = ctx.enter_context(tc.tile_pool(name="wp", bufs=1))
        ln_p = ctx.enter_context(tc.tile_pool(name="lnp", bufs=2))
        xnT_p = ctx.enter_context(tc.tile_pool(name="xnt", bufs=1))
        ae_p = ctx.enter_context(tc.tile_pool(name="aep", bufs=2))
        bc_p = ctx.enter_context(tc.tile_pool(name="bcp", bufs=2))
        fo_p = ctx.enter_context(tc.tile_pool(name="fout", bufs=2))

        # psum (8 banks): big 2 tags x [128,1024]f32 (4) + po 2 bufs [128,512]
        # (2) + fin 2 bufs [128,512] (2)
        psum_big = ctx.enter_context(tc.tile_pool(name="psbig", bufs=1, space="PSUM"))
        psum_o = ctx.enter_context(tc.tile_pool(name="pso", bufs=2, space="PSUM"))
        psum_f = ctx.enter_context(tc.tile_pool(name="psf", bufs=2, space="PSUM"))
        bigctr = [0]

        def big_tile(name):
            t = psum_big.tile([128, 2 * QB], F32, name=name,
                              tag=f"big{bigctr[0] % 2}")
            bigctr[0] += 1
            return t

        # ---------------- constants ----------------
        identb = const_p.tile([128, 128], BF16, name="identb")
        make_identity(nc, identb)

        eps_t = const_p.tile([128, 1], F32, name="eps_t")
        nc.vector.memset(eps_t, EPS)

        lnw_bc = const_p.tile([1, DIM], F32, name="lnw_bc")
        lnb_bc = const_p.tile([1, DIM], F32, name="lnb_bc")
        bout_bc = const_p.tile([128, DIM], F32, name="bout_bc")
        nc.gpsimd.dma_start(out=lnw_bc, in_=ln_w)
        nc.gpsimd.dma_start(out=lnb_bc, in_=ln_b)
        nc.gpsimd.dma_start(out=bout_bc, in_=b_out.to_broadcast([128, DIM]))

        ones8 = const_p.tile([128, 8], BF16, name="ones8")
        nc.vector.memset(ones8, 1.0)

        # ln_w / ln_b as feature-major columns: lnwc[p, c] = ln_w[128c + p]
        # (transpose [1,128] slices via PE, fp32)
        lnwc = const_p.tile([128, NC_], F32, name="lnwc")
        lnbc = const_p.tile([128, NC_], F32, name="lnbc")
        identf = const_p.tile([1, 1], F32, name="identf")
        nc.vector.memset(identf, 1.0)
        for c in range(NC_):
            for colt, bc_src in ((lnwc, lnw_bc), (lnbc, lnb_bc)):
                pcol = psum_f.tile([128, 1], F32, name="pcol", tag="fin")
                nc.tensor.transpose(out=pcol, in_=bc_src[0:1, c * 128:(c + 1) * 128],
                                    identity=identf)
                nc.vector.tensor_copy(out=colt[:, c:c + 1], in_=pcol)

        # ---------------- weights (bf16, stream early) ----------------
        wq = []
        for c in range(NC_):
            t = w_p.tile([128, 3 * INNER], BF16, name=f"wq{c}", tag=f"wq{c}")
            nc.sync.dma_start(out=t, in_=w_qkv[c * 128:(c + 1) * 128, :])
            wq.append(t)
        wt = []
        for c in range(NC_):
            t = w_p.tile([128, 2 * INNER], BF16, name=f"wt{c}", tag=f"wt{c}")
            nc.gpsimd.dma_start(out=t, in_=w_tab[c * 128:(c + 1) * 128, :])
            wt.append(t)
        wo = []
        for c in range(NC_):
            t = w_p.tile([128, DIM], BF16, name=f"wo{c}", tag=f"wo{c}")
            nc.gpsimd.dma_start(out=t, in_=w_out[c * 128:(c + 1) * 128, :])
            wo.append(t)

        # ---------------- persistent activations ----------------
        xnT = [xnT_p.tile([128, NTOK], BF16, name=f"xnT{c}", tag=f"xnT{c}")
               for c in range(NC_)]
        qT = [qkT_p.tile([128, NTOK], BF16, name=f"qT{c}", tag=f"qT{c}") for c in range(NC_)]
        kT = [qkT_p.tile([128, NTOK], BF16, name=f"kT{c}", tag=f"kT{c}") for c in range(NC_)]
        # v token-major with interleaved ones column per head: 8 x (64+1) = 520
        v_sb = [v_p.tile([128, 520], BF16, name=f"v{t}", tag=f"v{t}") for t in range(NT)]
        # unnormalized out^T chunks [128, 1024] (bf16)
        outuT = [outuT_p.tile([128, NTOK], BF16, name=f"ouT{c}", tag=f"ouT{c}")
                 for c in range(NC_)]

        # tab small tiles
        tnT = small_p.tile([128, NC_], BF16, name="tnT")      # tab LN^T columns
        k_tT = small_p.tile([128, NC_], BF16, name="k_tT")    # tab key, feat-major
        v_tT = small_p.tile([128, NC_], BF16, name="v_tT")    # tab value, feat-major
        kpad = small_p.tile([128, 32], BF16, name="kpad")     # zero-padded key blocks
        vpad = small_p.tile([128, 32], BF16, name="vpad")     # zero-padded value blocks
        W_vt = small_p.tile([8, DIM], BF16, name="W_vt")      # v_t @ w_out rows
        # packed [h, qb*512] layouts (8 rows)
        tabexp8 = small_p.tile([8, NTOK], F32, name="tabexp8")  # exp(tab dots)
        rcp8 = small_p.tile([8, NTOK], F32, name="rcp8")
        na8 = small_p.tile([8, NTOK], BF16, name="na8")       # normalized tab weights
        # strided staging [32*(h%4), (h//4)*1024 + qb*512] (quadrant-legal
        # targets for DVE moves out of PSUM partition 64)
        tabstr = small_p.tile([128, 2 * NTOK], F32, name="tabstr")
        dall = small_p.tile([128, 2 * NTOK], F32, name="dall")
        rcp_s = small_p.tile([128, 2 * NTOK], F32, name="rcp_s")
        rcpb_s = small_p.tile([128, 2 * NTOK], BF16, name="rcpb_s")
        nc.vector.memset(dall, 1.0)
        nc.vector.memset(tabstr, 1.0)

        def slot(h):
            return 32 * (h % 4), (h // 4) * NTOK  # (row, col block base)

        # ---------------- phase 1: img LN + transpose ----------------
        x_ts = []
        for t in range(NT):
            x_t = ln_p.tile([128, DIM], F32, name="x_t", tag="x_t", bufs=4)
            nc.sync.dma_start(out=x_t, in_=img[t * 128:(t + 1) * 128, :])
            x_ts.append(x_t)

        for t in range(NT):
            x_t = x_ts[t]
            stats = ln_p.tile([128, 6], F32, name="stats", tag="stats")
            nc.vector.bn_stats(out=stats, in_=x_t)
            mv = ln_p.tile([128, 2], F32, name="mv", tag="mv")
            nc.vector.bn_aggr(out=mv, in_=stats)
            sd = ln_p.tile([128, 1], F32, name="sd", tag="sd")
            nc.scalar.activation(out=sd, in_=mv[:, 1:2], func=AF.Sqrt,
                                 bias=eps_t, scale=1.0)
            rstd = ln_p.tile([128, 1], F32, name="rstd", tag="rstd")
            nc.vector.reciprocal(out=rstd, in_=sd)

            xn_t = ln_p.tile([128, DIM], BF16, name="xn_t", tag="xn_t", bufs=4)
            nc.vector.tensor_scalar(out=xn_t, in0=x_t,
                                    scalar1=mv[:, 0:1], scalar2=rstd,
                                    op0=OP.subtract, op1=OP.mult)
            # transpose 4 chunks into one bf16 psum tile, apply ln_w/ln_b
            ptb = big_tile("pt").bitcast(BF16)  # [128, 2048] bf16 view
            for c in range(NC_):
                nc.tensor.transpose(out=ptb[:, c * 128:(c + 1) * 128],
                                    in_=xn_t[:, c * 128:(c + 1) * 128],
                                    identity=identb)
            for c in range(NC_):
                nc.vector.tensor_scalar(
                    out=xnT[c][:, t * 128:(t + 1) * 128],
                    in0=ptb[:, c * 128:(c + 1) * 128],
                    scalar1=lnwc[:, c:c + 1], scalar2=lnbc[:, c:c + 1],
                    op0=OP.mult, op1=OP.add)

        # ---------------- tab LN (1 row) + tnT ----------------
        tb = ln_p.tile([1, DIM], F32, name="tb", tag="tb", bufs=1)
        nc.sync.dma_start(out=tb, in_=tab)
        tstats = ln_p.tile([1, 6], F32, name="tstats", tag="tstats")
        nc.vector.bn_stats(out=tstats, in_=tb)
        tmv = ln_p.tile([1, 2], F32, name="tmv", tag="tmv")
        nc.vector.bn_aggr(out=tmv, in_=tstats)
        tsd = ln_p.tile([1, 1], F32, name="tsd", tag="tsd")
        nc.scalar.activation(out=tsd, in_=tmv[:, 1:2], func=AF.Sqrt,
                             bias=eps_t[0:1], scale=1.0)
        trstd = ln_p.tile([1, 1], F32, name="trstd", tag="trstd")
        nc.vector.reciprocal(out=trstd, in_=tsd)
        tn = ln_p.tile([1, DIM], F32, name="tn", tag="tn", bufs=1)
        nc.vector.tensor_scalar(out=tn, in0=tb, scalar1=tmv[:, 0:1],
                                scalar2=trstd, op0=OP.subtract, op1=OP.mult)
        nc.vector.tensor_tensor(out=tn, in0=tn, in1=lnw_bc, op=OP.mult)
        tnb = ln_p.tile([1, DIM], BF16, name="tnb", tag="tnb", bufs=1)
        nc.vector.tensor_tensor(out=tnb, in0=tn, in1=lnb_bc, op=OP.add)
        for c in range(NC_):
            pt = psum_f.tile([128, 1], BF16, name="ptn", tag="fin")
            nc.tensor.transpose(out=pt, in_=tnb[0:1, c * 128:(c + 1) * 128],
                                identity=identb[0:1, 0:1])
            nc.vector.tensor_copy(out=tnT[:, c:c + 1], in_=pt)

        # tab k/v (feature-major cols + padded blocks)
        nc.vector.memset(kpad, 0.0)
        nc.vector.memset(vpad, 0.0)
        for c in range(NC_):
            ps = psum_f.tile([128, 1], F32, name="pskt", tag="fin")
            for kc in range(NC_):
                nc.tensor.matmul(
                    ps,
                    lhsT=wt[kc][:, c * 128:(c + 1) * 128],
                    rhs=tnT[:, kc:kc + 1],
                    start=(kc == 0), stop=(kc == NC_ - 1))
            nc.vector.tensor_copy(out=k_tT[:, c:c + 1], in_=ps)
        ps_vt = psum_f.tile([1, INNER], F32, name="psvt", tag="fin")
        for kc in range(NC_):
            nc.tensor.matmul(
                ps_vt,
                lhsT=tnT[:, kc:kc + 1],
                rhs=wt[kc][:, INNER:2 * INNER],
                start=(kc == 0), stop=(kc == NC_ - 1))
        vt_b = ln_p.tile([1, INNER], BF16, name="vt_b", tag="vt_b", bufs=1)
        nc.vector.tensor_copy(out=vt_b, in_=ps_vt)
        for c in range(NC_):
            pt = psum_f.tile([128, 1], BF16, name="ptv", tag="fin")
            nc.tensor.transpose(out=pt, in_=vt_b[0:1, c * 128:(c + 1) * 128],
                                identity=identb[0:1, 0:1])
            nc.vector.tensor_copy(out=v_tT[:, c:c + 1], in_=pt)
        # scatter into zero-padded blocks: head h = 2c+j lives at
        # col (8c + h) rows 64j:64j+64 of kpad/vpad
        for c in range(NC_):
            for j in range(2):
                h = 2 * c + j
                nc.vector.tensor_copy(
                    out=kpad[64 * j:64 * j + 64, 8 * c + h:8 * c + h + 1],
                    in_=k_tT[64 * j:64 * j + 64, c:c + 1])
                nc.vector.tensor_copy(
                    out=vpad[64 * j:64 * j + 64, 8 * c + h:8 * c + h + 1],
                    in_=v_tT[64 * j:64 * j + 64, c:c + 1])

        # ---------------- phase 2+3: qkT, dots, v, attn@v, final ----------
        def emit_qkT(hp):
            # qT[hp] then kT[hp]: feature-major qkv for head pair hp
            for m in (hp, hp + 4):
                dst = qT[m] if m < 4 else kT[m - 4]
                ps = big_tile("psqk")
                for qb in range(NQB):
                    half = ps[:, qb * QB:(qb + 1) * QB]
                    for kc in range(NC_):
                        nc.tensor.matmul(
                            half,
                            lhsT=wq[kc][:, m * 128:(m + 1) * 128],
                            rhs=xnT[kc][:, qb * QB:(qb + 1) * QB],
                            start=(kc == 0), stop=(kc == NC_ - 1))
                nc.vector.tensor_copy(out=dst, in_=ps)

        ae_tiles = {}

        def emit_dots(hp, qb):
            # dots^T + exp for both heads of pair hp, q block qb
            ae0 = ae_p.tile([128, 8 * QB], BF16, name="ae0", tag="ae0")
            ae1 = ae_p.tile([128, 8 * QB], BF16, name="ae1", tag="ae1")
            ae_tiles[(hp, qb)] = (ae0, ae1)
            for kp in range(4):   # kpos chunk pairs
                ps0 = big_tile("psd0")
                ps1 = big_tile("psd1")
                for i, kt in enumerate((2 * kp, 2 * kp + 1)):
                    for hh, ps in ((0, ps0), (1, ps1)):
                        hb = hh * 64
                        nc.tensor.matmul(
                            ps[:, i * QB:(i + 1) * QB],
                            lhsT=kT[hp][hb:hb + 64, kt * 128:(kt + 1) * 128],
                            rhs=qT[hp][hb:hb + 64, qb * QB:(qb + 1) * QB],
                            start=True, stop=True)
                nc.scalar.activation(
                    out=ae0[:, 2 * kp * QB:(2 * kp + 2) * QB],
                    in_=ps0, func=AF.Exp, scale=SCALE)
                nc.scalar.activation(
                    out=ae1[:, 2 * kp * QB:(2 * kp + 2) * QB],
                    in_=ps1, func=AF.Exp, scale=SCALE)

        def emit_attnv(hp, qb):
            ae0, ae1 = ae_tiles.pop((hp, qb))
            qs = slice(qb * QB, (qb + 1) * QB)
            for hh, ae in ((0, ae0), (1, ae1)):
                h = 2 * hp + hh
                po = psum_o.tile([128, QB], F32, name="po", tag="o")
                for kt in range(NT):
                    nc.tensor.matmul(
                        po[0:65, :],
                        lhsT=v_sb[kt][:, 65 * h:65 * h + 65],
                        rhs=ae[:, kt * QB:(kt + 1) * QB],
                        start=(kt == 0), stop=(kt == NT - 1))
                nc.vector.tensor_copy(
                    out=outuT[hp][64 * hh:64 * hh + 64, qs],
                    in_=po[0:64, :])
                r, gc = slot(h)
                nc.vector.tensor_copy(
                    out=dall[r:r + 1, gc + qb * QB:gc + (qb + 1) * QB],
                    in_=po[64:65, :])

        def emit_normalize(qb):
            qs = slice(qb * QB, (qb + 1) * QB)
            for g in range(2):
                cs = slice(g * NTOK + qb * QB, g * NTOK + (qb + 1) * QB)
                nc.vector.tensor_tensor(out=dall[0:97, cs], in0=dall[0:97, cs],
                                        in1=tabstr[0:97, cs], op=OP.add)
                nc.vector.reciprocal_approx_fast(out=rcp_s[0:97, cs],
                                                 in_=dall[0:97, cs])
                nc.vector.tensor_copy(out=rcpb_s[0:97, cs], in_=rcp_s[0:97, cs])
            # repack recip rows for the tab K=8 row of the final matmul
            for h in range(HEADS):
                r, gc = slot(h)
                nc.sync.dma_start(
                    out=rcp8[h:h + 1, qs],
                    in_=rcp_s[r:r + 1, gc + qb * QB:gc + (qb + 1) * QB])
            nc.vector.tensor_tensor(out=na8[:, qs], in0=tabexp8[:, qs],
                                    in1=rcp8[:, qs], op=OP.mult)
            for h in range(HEADS):
                r, gc = slot(h)
                nc.sync.dma_start(
                    out=dscr[h:h + 1, qs],
                    in_=rcpb_s[r:r + 1, gc + qb * QB:gc + (qb + 1) * QB])
            for hp in range(4):
                bc = bc_p.tile([128, QB], BF16, name="bc", tag=f"bc{hp % 2}")
                nc.sync.dma_start(
                    out=bc[0:64, :],
                    in_=dscr[2 * hp:2 * hp + 1, qs].to_broadcast([64, QB]))
                nc.sync.dma_start(
                    out=bc[64:128, :],
                    in_=dscr[2 * hp + 1:2 * hp + 2, qs].to_broadcast([64, QB]))
                nc.vector.tensor_tensor(
                    out=outuT[hp][:, qs], in0=outuT[hp][:, qs],
                    in1=bc, op=OP.mult)

        def emit_final(t):
            pf = psum_f.tile([128, DIM], F32, name="pf", tag="fin")
            for c in range(NC_):
                nc.tensor.matmul(
                    pf,
                    lhsT=outuT[c][:, t * 128:(t + 1) * 128],
                    rhs=wo[c],
                    start=(c == 0), stop=False)
            nc.tensor.matmul(
                pf,
                lhsT=na8[0:8, t * 128:(t + 1) * 128],
                rhs=W_vt,
                start=False, stop=True)
            fo = fo_p.tile([128, DIM], F32, name="fo", tag="fo")
            nc.vector.tensor_tensor(out=fo, in0=pf, in1=bout_bc, op=OP.add)
            nc.sync.dma_start(out=out_d[t * 128:(t + 1) * 128, :], in_=fo)

        # --- PE issue order ---
        emit_qkT(0)
        emit_dots(0, 0)
        emit_qkT(1)
        emit_dots(1, 0)
        emit_qkT(2)
        emit_qkT(3)

        # tab dots: one packed M=8 matmul chain per q block + single exp
        for qb in range(NQB):
            ptab = psum_o.tile([8, QB], F32, name="ptab", tag="o")
            for c in range(NC_):
                nc.tensor.matmul(
                    ptab,
                    lhsT=kpad[:, 8 * c:8 * c + 8],
                    rhs=qT[c][:, qb * QB:(qb + 1) * QB],
                    start=(c == 0), stop=(c == NC_ - 1))
            nc.scalar.activation(
                out=tabexp8[:, qb * QB:(qb + 1) * QB],
                in_=ptab, func=AF.Exp, scale=SCALE)
        # repack exp(tab dots) into the strided staging layout for the
        # denominator add (DMA moves rows to arbitrary partitions)
        for h in range(HEADS):
            r, gc = slot(h)
            nc.gpsimd.dma_start(
                out=tabstr[r:r + 1, gc:gc + NTOK],
                in_=tabexp8[h:h + 1, :])

        # W_vt = v_t @ w_out (K=8-packed blocks against wo chunks)
        psW = psum_f.tile([8, DIM], F32, name="psW", tag="fin")
        for c in range(NC_):
            nc.tensor.matmul(
                psW,
                lhsT=vpad[:, 8 * c:8 * c + 8],
                rhs=wo[c],
                start=(c == 0), stop=(c == NC_ - 1))
        nc.vector.tensor_copy(out=W_vt, in_=psW)

        # v token-major (+ ones interleave)
        for t in range(NT):
            ps = big_tile("psv")
            pv = ps[:, 0:QB]
            for kc in range(NC_):
                nc.tensor.matmul(
                    pv,
                    lhsT=xnT[kc][:, t * 128:(t + 1) * 128],
                    rhs=wq[kc][:, 2 * INNER:3 * INNER],
                    start=(kc == 0), stop=(kc == NC_ - 1))
            vdst = v_sb[t].rearrange("p (h s) -> p h s", s=65)
            nc.vector.tensor_copy(out=vdst[:, :, 0:64],
                                  in_=pv.rearrange("p (h d) -> p h d", d=64))
            nc.vector.tensor_copy(
                out=vdst[:, :, 64:65],
                in_=ones8.rearrange("p (h o) -> p h o", o=1))

        # attention steady state (attn@v interleaved with next dots)
        emit_attnv(0, 0); emit_dots(2, 0)
        emit_attnv(1, 0); emit_dots(3, 0)
        emit_attnv(2, 0); emit_dots(0, 1)
        emit_attnv(3, 0); emit_dots(1, 1)
        emit_normalize(0)
        emit_attnv(0, 1); emit_dots(2, 1)
        emit_final(0); emit_final(1)
        emit_attnv(1, 1); emit_dots(3, 1)
        emit_final(2); emit_final(3)
        emit_attnv(2, 1)
        emit_attnv(3, 1)
        emit_normalize(1)
        for t in range(4, 8):
            emit_final(t)


_CACHED_NC = None


def _to_bf16(a):
    import ml_dtypes
    return np.ascontiguousarray(np.asarray(a, dtype=np.float32)).astype(
        ml_dtypes.bfloat16)


def kernel(**inputs):
    global _CACHED_NC
    img = np.ascontiguousarray(np.asarray(inputs["img"], dtype=np.float32))
    tab = np.ascontiguousarray(np.asarray(inputs["tab"], dtype=np.float32))
    w_qkv = _to_bf16(inputs["w_qkv"])
    w_tab_qkv = np.asarray(inputs["w_tab_qkv"], dtype=np.float32)
    w_tab = _to_bf16(w_tab_qkv[:, INNER:3 * INNER])
    w_out = _to_bf16(inputs["w_out"])
    b_out = np.asarray(inputs["b_out"], dtype=np.float32).reshape(1, DIM)
    ln_w = np.asarray(inputs["ln_w"], dtype=np.float32).reshape(1, DIM)
    ln_b = np.asarray(inputs["ln_b"], dtype=np.float32).reshape(1, DIM)

    if _CACHED_NC is None:
        _CACHED_NC = build_program()
    nc = _CACHED_NC

    in_maps = []
    for b in range(N_CORES):
        in_maps.append({
            "img_s": np.ascontiguousarray(img[b]),
            "tab_s": np.ascontiguousarray(tab[b]),
            "wq_b": w_qkv,
            "wt_b": w_tab,
            "wo_b": w_out,
            "b_out": b_out,
            "ln_w": ln_w,
            "ln_b": ln_b,
        })

    res = bass_utils.run_bass_kernel_spmd(nc, in_maps, core_ids=list(range(N_CORES)))
    out = np.stack([res.results[c]["out_s"] for c in range(N_CORES)], axis=0)
    return out.astype(np.float32)


if __name__ == "__main__":
    d = np.load("/root/problem/ref_data.npz")
    ins = {k: d[k] for k in ("img", "tab", "w_qkv", "w_tab_qkv", "w_out",
                             "b_out", "ln_w", "ln_b")}
    actual = kernel(**ins)
    expected = d["expected"]
    err = np.abs(actual - expected).max()
    rel = err / np.abs(expected).max()
    print("absmax err:", err, "rel:", rel)


# revision 27
# speedup vs baseline: 1.0804x; 1.0804x over previous
"""Trainium2 Bass kernel for nn_Attention_43868795961547 (sparse_attention).

Reference computation per batch item (8 items, data-parallel over 8 cores):
  x  = LN(img[b]) @ w_qkv -> q,k,v (8 heads x 64)          [1024 tokens]
  kt,vt from LN(tab[b]) @ w_tab_qkv appended as key/value position 1024
  out = softmax(q k^T / 8) @ v ; out @ w_out + b_out        -> [1024, 512]

v2 strategy (per core), all matmuls bf16 (1-pass PE + FWL weight loads):
  - LN stats fp32 (bn_stats), xn cast bf16, PE-transposed; ln_w/ln_b applied
    as per-partition scale/bias in the PSUM->SBUF move (DVE).
  - qT,kT feature-major [512 x 1024] bf16; v token-major with per-head
    interleaved ones column (65-col groups) so attn@v emits softmax
    denominators for free.
  - tab token: k_t dots for all 8 heads in ONE packed M=8 matmul chain
    (zero-padded block lhsT), exp'd once; tab's rank-1 contribution to the
    output folded into the final projection as a K=8 accumulation row
    (lhsT = normalized tab weights, rhs = v_t @ w_out precomputed on PE).
  - dots^T[kpos, q] per head pair row-packed (K=64 tiles at rows 0/64);
    exp on ACT from PSUM (scale=1/8 folded), ae bf16.
  - attn@v: out^T[65, q] over 8 kpos chunks; denominators DMA-packed to
    rows 0..7, reciprocal_approx_fast, gpsimd partition_broadcast for the
    per-head normalize multiply.
  - PE issue order interleaves attn@v(hp) with dots(hp') so the PE rides
    the ACT exp pipeline without starving.
"""

import numpy as np

import concourse.bass as bass
import concourse.mybir as mybir
import concourse.tile as tile
from concourse import bacc
from concourse import bass_utils
from concourse.masks import make_identity

F32 = mybir.dt.float32
BF16 = mybir.dt.bfloat16

N_CORES = 8
NTOK = 1024  # img tokens per batch item
DIM = 512
HEADS = 8
DHEAD = 64
INNER = 512
SCALE = DHEAD ** -0.5  # 0.125
EPS = 1e-5

NT = NTOK // 128   # 8 token tiles
NC_ = DIM // 128   # 4 feature chunks
NQB = 2            # q blocks of 512
QB = 512


def build_program():
    nc = bacc.Bacc(
        "TRN2",
        target_bir_lowering=False,
        debug=False,
        enable_asserts=False,
        num_devices=N_CORES,
    )

    img = nc.dram_tensor("img_s", [NTOK, DIM], F32, kind="ExternalInput").ap()
    tab = nc.dram_tensor("tab_s", [1, DIM], F32, kind="ExternalInput").ap()
    w_qkv = nc.dram_tensor("wq_b", [DIM, 3 * INNER], BF16, kind="ExternalInput").ap()
    # only k,v columns of w_tab_qkv (cols 512:1536) are used
    w_tab = nc.dram_tensor("wt_b", [DIM, 2 * INNER], BF16, kind="ExternalInput").ap()
    w_out = nc.dram_tensor("wo_b", [INNER, DIM], BF16, kind="ExternalInput").ap()
    b_out = nc.dram_tensor("b_out", [1, DIM], F32, kind="ExternalInput").ap()
    ln_w = nc.dram_tensor("ln_w", [1, DIM], F32, kind="ExternalInput").ap()
    ln_b = nc.dram_tensor("ln_b", [1, DIM], F32, kind="ExternalInput").ap()
    out_d = nc.dram_tensor("out_s", [NTOK, DIM], F32, kind="ExternalOutput").ap()
    dscr = nc.dram_tensor("dscratch", [HEADS, NTOK], BF16, kind="Internal").ap()

    with tile.TileContext(nc) as tc:
        kernel_body(tc, img, tab, w_qkv, w_tab, w_out, b_out, ln_w, ln_b, out_d,
                    dscr)

    nc.compile()
    return nc


def kernel_body(tc, img, tab, w_qkv, w_tab, w_out, b_out, ln_w, ln_b, out_d,
                dscr):
    nc = tc.nc
    AF = mybir.ActivationFunctionType
    OP = mybir.AluOpType

    import contextlib
    ctx = contextlib.ExitStack()
    with ctx:
        # ---------------- pools ----------------
        const_p = ctx.enter_context(tc.tile_pool(name="const", bufs=1))
        qkT_p = ctx.enter_context(tc.tile_pool(name="qkT", bufs=1))
        v_p = ctx.enter_context(tc.tile_pool(name="vp", bufs=1))
        outuT_p = ctx.enter_context(tc.tile_pool(name="outuT", bufs=1))
        small_p = ctx.enter_context(tc.tile_pool(name="smallp", bufs=1))
        w_p = ctx.enter_context(tc.tile_pool(name="wp", bufs=1))
        ln_p = ctx.enter_context(tc.tile_pool(name="lnp", bufs=2))
        xnT_p = ctx.enter_context(tc.tile_pool(name="xnt", bufs=1))
        ae_p = ctx.enter_context(tc.tile_pool(name="aep", bufs=2))
        bc_p = ctx.enter_context(tc.tile_pool(name="bcp", bufs=2))
        fo_p = ctx.enter_context(tc.tile_pool(name="fout", bufs=2))

        # psum (8 banks): big 2 tags x [128,1024]f32 (4) + po 2 bufs [128,512]
        # (2) + fin 2 bufs [128,512] (2)
        psum_big = ctx.enter_context(tc.tile_pool(name="psbig", bufs=1, space="PSUM"))
        psum_o = ctx.enter_context(tc.tile_pool(name="pso", bufs=2, space="PSUM"))
        psum_f = ctx.enter_context(tc.tile_pool(name="psf", bufs=2, space="PSUM"))
        bigctr = [0]

        def big_tile(name):
            t = psum_big.tile([128, 2 * QB], F32, name=name,
                              tag=f"big{bigctr[0] % 2}")
            bigctr[0] += 1
            return t

        # ---------------- constants ----------------
        identb = const_p.tile([128, 128], BF16, name="identb")
        make_identity(nc, identb)

        eps_t = const_p.tile([128, 1], F32, name="eps_t")
        nc.vector.memset(eps_t, EPS)

        lnw_bc = const_p.tile([1, DIM], F32, name="lnw_bc")
        lnb_bc = const_p.tile([1, DIM], F32, name="lnb_bc")
        bout_bc = const_p.tile([128, DIM], F32, name="bout_bc")
        nc.sync.dma_start(out=lnw_bc, in_=ln_w)
        nc.sync.dma_start(out=lnb_bc, in_=ln_b)
        nc.gpsimd.dma_start(out=bout_bc, in_=b_out.to_broadcast([128, DIM]))

        ones8 = const_p.tile([128, 8], BF16, name="ones8")
        nc.vector.memset(ones8, 1.0)

        # ln_w / ln_b as feature-major columns: lnwc[p, c] = ln_w[128c + p]
        # (transpose [1,128] slices via PE, fp32)
        lnwc = const_p.tile([128, NC_], F32, name="lnwc")
        lnbc = const_p.tile([128, NC_], F32, name="lnbc")
        identf = const_p.tile([1, 1], F32, name="identf")
        nc.vector.memset(identf, 1.0)
        for c in range(NC_):
            for colt, bc_src in ((lnwc, lnw_bc), (lnbc, lnb_bc)):
                pcol = psum_f.tile([128, 1], F32, name="pcol", tag="fin")
                nc.tensor.transpose(out=pcol, in_=bc_src[0:1, c * 128:(c + 1) * 128],
                                    identity=identf)
                nc.vector.tensor_copy(out=colt[:, c:c + 1], in_=pcol)

        # ---------------- input/weight DMA (img first: LN gates the PE) ----
        x_ts = []
        for t in range(NT):
            x_t = ln_p.tile([128, DIM], F32, name="x_t", tag="x_t", bufs=8)
            nc.sync.dma_start(out=x_t, in_=img[t * 128:(t + 1) * 128, :])
            x_ts.append(x_t)
        tb = ln_p.tile([1, DIM], F32, name="tb", tag="tb", bufs=1)
        nc.gpsimd.dma_start(out=tb, in_=tab)

        wq = []
        for c in range(NC_):
            t = w_p.tile([128, 3 * INNER], BF16, name=f"wq{c}", tag=f"wq{c}")
            nc.sync.dma_start(out=t, in_=w_qkv[c * 128:(c + 1) * 128, :])
            wq.append(t)
        wt = []
        for c in range(NC_):
            t = w_p.tile([128, 2 * INNER], BF16, name=f"wt{c}", tag=f"wt{c}")
            nc.gpsimd.dma_start(out=t, in_=w_tab[c * 128:(c + 1) * 128, :])
            wt.append(t)
        wo = []
        for c in range(NC_):
            t = w_p.tile([128, DIM], BF16, name=f"wo{c}", tag=f"wo{c}")
            nc.gpsimd.dma_start(out=t, in_=w_out[c * 128:(c + 1) * 128, :])
            wo.append(t)

        # ---------------- persistent activations ----------------
        xnT = [xnT_p.tile([128, NTOK], BF16, name=f"xnT{c}", tag=f"xnT{c}")
               for c in range(NC_)]
        qT = [qkT_p.tile([128, NTOK], BF16, name=f"qT{c}", tag=f"qT{c}") for c in range(NC_)]
        kT = [qkT_p.tile([128, NTOK], BF16, name=f"kT{c}", tag=f"kT{c}") for c in range(NC_)]
        # v token-major with interleaved ones column per head: 8 x (64+1) = 520
        v_sb = [v_p.tile([128, 520], BF16, name=f"v{t}", tag=f"v{t}") for t in range(NT)]
        # unnormalized out^T chunks [128, 1024] (bf16)
        outuT = [outuT_p.tile([128, NTOK], BF16, name=f"ouT{c}", tag=f"ouT{c}")
                 for c in range(NC_)]

        # tab small tiles
        tnT = small_p.tile([128, NC_], BF16, name="tnT")      # tab LN^T columns
        k_tT = small_p.tile([128, NC_], BF16, name="k_tT")    # tab key, feat-major
        v_tT = small_p.tile([128, NC_], BF16, name="v_tT")    # tab value, feat-major
        kpad = small_p.tile([128, 32], BF16, name="kpad")     # zero-padded key blocks
        vpad = small_p.tile([128, 32], BF16, name="vpad")     # zero-padded value blocks
        W_vt = small_p.tile([8, DIM], BF16, name="W_vt")      # v_t @ w_out rows
        # packed [h, qb*512] layouts (8 rows)
        tabexp8 = small_p.tile([8, NTOK], F32, name="tabexp8")  # exp(tab dots)
        rcp8 = small_p.tile([8, NTOK], F32, name="rcp8")
        na8 = small_p.tile([8, NTOK], BF16, name="na8")       # normalized tab weights
        # strided staging [32*(h%4), (h//4)*1024 + qb*512] (quadrant-legal
        # targets for DVE moves out of PSUM partition 64)
        tabstr = small_p.tile([128, 2 * NTOK], F32, name="tabstr")
        dall = small_p.tile([128, 2 * NTOK], F32, name="dall")
        rcp_s = small_p.tile([128, 2 * NTOK], F32, name="rcp_s")
        rcpb_s = small_p.tile([128, 2 * NTOK], BF16, name="rcpb_s")
        # zero-init so full-span [0:97] ops never read uninitialized rows
        # (slot rows are overwritten; other lanes carry junk that is never
        # consumed). On ACT: DVE is the critical engine during the front.
        nc.gpsimd.memset(dall, 1.0)
        nc.scalar.memzero(tabstr)
        nc.scalar.memzero(rcp8)

        def slot(h):
            return 32 * (h % 4), (h // 4) * NTOK  # (row, col block base)

        # ---------------- phase 1: img LN + transpose ----------------
        for t in range(NT):
            x_t = x_ts[t]
            stats = ln_p.tile([128, 6], F32, name="stats", tag="stats")
            nc.vector.bn_stats(out=stats, in_=x_t)
            mv = ln_p.tile([128, 2], F32, name="mv", tag="mv")
            nc.vector.bn_aggr(out=mv, in_=stats)
            sd = ln_p.tile([128, 1], F32, name="sd", tag="sd")
            nc.scalar.activation(out=sd, in_=mv[:, 1:2], func=AF.Sqrt,
                                 bias=eps_t, scale=1.0)
            rstd = ln_p.tile([128, 1], F32, name="rstd", tag="rstd")
            nc.vector.reciprocal(out=rstd, in_=sd)

            xn_t = ln_p.tile([128, DIM], BF16, name="xn_t", tag="xn_t", bufs=4)
            nc.vector.tensor_scalar(out=xn_t, in0=x_t,
                                    scalar1=mv[:, 0:1], scalar2=rstd,
                                    op0=OP.subtract, op1=OP.mult)
            # transpose 4 chunks into one bf16 psum tile, apply ln_w/ln_b
            ptb = big_tile("pt").bitcast(BF16)  # [128, 2048] bf16 view
            for c in range(NC_):
                nc.tensor.transpose(out=ptb[:, c * 128:(c + 1) * 128],
                                    in_=xn_t[:, c * 128:(c + 1) * 128],
                                    identity=identb)
            # ln affine fused into the PSUM->SBUF move, on ACT (idle in front)
            for c in range(NC_):
                nc.scalar.activation(
                    out=xnT[c][:, t * 128:(t + 1) * 128],
                    in_=ptb[:, c * 128:(c + 1) * 128],
                    func=AF.Identity,
                    scale=lnwc[:, c:c + 1], bias=lnbc[:, c:c + 1])

        # ---------------- tab LN (1 row) + tnT ----------------
        tstats = ln_p.tile([1, 6], F32, name="tstats", tag="tstats")
        nc.vector.bn_stats(out=tstats, in_=tb)
        tmv = ln_p.tile([1, 2], F32, name="tmv", tag="tmv")
        nc.vector.bn_aggr(out=tmv, in_=tstats)
        tsd = ln_p.tile([1, 1], F32, name="tsd", tag="tsd")
        nc.scalar.activation(out=tsd, in_=tmv[:, 1:2], func=AF.Sqrt,
                             bias=eps_t[0:1], scale=1.0)
        trstd = ln_p.tile([1, 1], F32, name="trstd", tag="trstd")
        nc.vector.reciprocal(out=trstd, in_=tsd)
        tn = ln_p.tile([1, DIM], F32, name="tn", tag="tn", bufs=1)
        nc.vector.tensor_scalar(out=tn, in0=tb, scalar1=tmv[:, 0:1],
                                scalar2=trstd, op0=OP.subtract, op1=OP.mult)
        nc.vector.tensor_tensor(out=tn, in0=tn, in1=lnw_bc, op=OP.mult)
        tnb = ln_p.tile([1, DIM], BF16, name="tnb", tag="tnb", bufs=1)
        nc.vector.tensor_tensor(out=tnb, in0=tn, in1=lnb_bc, op=OP.add)
        for c in range(NC_):
            pt = psum_f.tile([128, 1], BF16, name="ptn", tag="fin")
            nc.tensor.transpose(out=pt, in_=tnb[0:1, c * 128:(c + 1) * 128],
                                identity=identb[0:1, 0:1])
            nc.vector.tensor_copy(out=tnT[:, c:c + 1], in_=pt)

        # tab k/v (feature-major cols + padded blocks) — emitted later in the
        # PE order so the PE isn't blocked on the wt/wo weight DMAs early.
        nc.gpsimd.memset(kpad, 0.0)
        nc.gpsimd.memset(vpad, 0.0)

        def emit_tab_kv():
            for c in range(NC_):
                ps = psum_f.tile([128, 1], F32, name="pskt", tag="fin")
                for kc in range(NC_):
                    nc.tensor.matmul(
                        ps,
                        lhsT=wt[kc][:, c * 128:(c + 1) * 128],
                        rhs=tnT[:, kc:kc + 1],
                        start=(kc == 0), stop=(kc == NC_ - 1))
                nc.vector.tensor_copy(out=k_tT[:, c:c + 1], in_=ps)
            ps_vt = psum_f.tile([1, INNER], F32, name="psvt", tag="fin")
            for kc in range(NC_):
                nc.tensor.matmul(
                    ps_vt,
                    lhsT=tnT[:, kc:kc + 1],
                    rhs=wt[kc][:, INNER:2 * INNER],
                    start=(kc == 0), stop=(kc == NC_ - 1))
            vt_b = ln_p.tile([1, INNER], BF16, name="vt_b", tag="vt_b", bufs=1)
            nc.vector.tensor_copy(out=vt_b, in_=ps_vt)
            for c in range(NC_):
                pt = psum_f.tile([128, 1], BF16, name="ptv", tag="fin")
                nc.tensor.transpose(out=pt, in_=vt_b[0:1, c * 128:(c + 1) * 128],
                                    identity=identb[0:1, 0:1])
                nc.vector.tensor_copy(out=v_tT[:, c:c + 1], in_=pt)
            # scatter into zero-padded blocks: head h = 2c+j lives at
            # col (8c + h) rows 64j:64j+64 of kpad/vpad
            for c in range(NC_):
                for j in range(2):
                    h = 2 * c + j
                    nc.vector.tensor_copy(
                        out=kpad[64 * j:64 * j + 64, 8 * c + h:8 * c + h + 1],
                        in_=k_tT[64 * j:64 * j + 64, c:c + 1])
                    nc.vector.tensor_copy(
                        out=vpad[64 * j:64 * j + 64, 8 * c + h:8 * c + h + 1],
                        in_=v_tT[64 * j:64 * j + 64, c:c + 1])

        # ---------------- phase 2+3: qkT, dots, v, attn@v, final ----------
        def emit_qkT(hp):
            # qT[hp] then kT[hp]: feature-major qkv for head pair hp
            for m in (hp, hp + 4):
                dst = qT[m] if m < 4 else kT[m - 4]
                ps = big_tile("psqk")
                for qb in range(NQB):
                    half = ps[:, qb * QB:(qb + 1) * QB]
                    for kc in range(NC_):
                        nc.tensor.matmul(
                            half,
                            lhsT=wq[kc][:, m * 128:(m + 1) * 128],
                            rhs=xnT[kc][:, qb * QB:(qb + 1) * QB],
                            start=(kc == 0), stop=(kc == NC_ - 1))
                nc.vector.tensor_copy(out=dst, in_=ps)

        ae_tiles = {}

        def emit_dots(hp, qb):
            # dots^T + exp for both heads of pair hp, q block qb
            ae0 = ae_p.tile([128, 8 * QB], BF16, name="ae0", tag="ae0")
            ae1 = ae_p.tile([128, 8 * QB], BF16, name="ae1", tag="ae1")
            ae_tiles[(hp, qb)] = (ae0, ae1)
            for kp in range(4):   # kpos chunk pairs
                ps0 = big_tile("psd0")
                ps1 = big_tile("psd1")
                for i, kt in enumerate((2 * kp, 2 * kp + 1)):
                    for hh, ps in ((0, ps0), (1, ps1)):
                        hb = hh * 64
                        nc.tensor.matmul(
                            ps[:, i * QB:(i + 1) * QB],
                            lhsT=kT[hp][hb:hb + 64, kt * 128:(kt + 1) * 128],
                            rhs=qT[hp][hb:hb + 64, qb * QB:(qb + 1) * QB],
                            start=True, stop=True)
                nc.scalar.activation(
                    out=ae0[:, 2 * kp * QB:(2 * kp + 2) * QB],
                    in_=ps0, func=AF.Exp, scale=SCALE)
                nc.scalar.activation(
                    out=ae1[:, 2 * kp * QB:(2 * kp + 2) * QB],
                    in_=ps1, func=AF.Exp, scale=SCALE)

        def emit_attnv(hp, qb):
            ae0, ae1 = ae_tiles.pop((hp, qb))
            qs = slice(qb * QB, (qb + 1) * QB)
            for hh, ae in ((0, ae0), (1, ae1)):
                h = 2 * hp + hh
                po = psum_o.tile([128, QB], F32, name="po", tag="o")
                for kt in range(NT):
                    nc.tensor.matmul(
                        po[0:65, :],
                        lhsT=v_sb[kt][:, 65 * h:65 * h + 65],
                        rhs=ae[:, kt * QB:(kt + 1) * QB],
                        start=(kt == 0), stop=(kt == NT - 1))
                nc.vector.tensor_copy(
                    out=outuT[hp][64 * hh:64 * hh + 64, qs],
                    in_=po[0:64, :])
                r, gc = slot(h)
                nc.vector.tensor_copy(
                    out=dall[r:r + 1, gc + qb * QB:gc + (qb + 1) * QB],
                    in_=po[64:65, :])

        def emit_norm_g(qb, g):
            # normalize heads 4g..4g+3 (= outuT pairs 2g, 2g+1) for q block qb
            qs = slice(qb * QB, (qb + 1) * QB)
            hs = slice(4 * g, 4 * g + 4)
            cs = slice(g * NTOK + qb * QB, g * NTOK + (qb + 1) * QB)
            nc.vector.tensor_tensor(out=dall[0:97, cs], in0=dall[0:97, cs],
                                    in1=tabstr[0:97, cs], op=OP.add)
            nc.vector.reciprocal_approx_fast(out=rcp_s[0:97, cs],
                                             in_=dall[0:97, cs])
            nc.vector.tensor_copy(out=rcpb_s[0:97, cs], in_=rcp_s[0:97, cs])
            # batched repacks of the 4 strided slot rows (DMA, gpsimd queue)
            str_rows_f = rcp_s.rearrange("(a b) q -> a b q", b=32)[0:4, 0:1, cs]
            str_rows_b = rcpb_s.rearrange("(a b) q -> a b q", b=32)[0:4, 0:1, cs]
            nc.gpsimd.dma_start(out=rcp8[hs, qs], in_=str_rows_f)
            nc.gpsimd.dma_start(out=dscr[hs, qs], in_=str_rows_b)
            # DVE start partition must be quadrant-aligned: span rows 0:8
            # (the other half recomputes with whatever rcp8 holds; the g=1
            # call runs last and leaves all 8 rows consistent)
            nc.vector.tensor_tensor(out=na8[:, qs], in0=tabexp8[:, qs],
                                    in1=rcp8[:, qs], op=OP.mult)
            for hp in (2 * g, 2 * g + 1):
                bc = bc_p.tile([128, QB], BF16, name="bc", tag=f"bc{hp % 2}")
                nc.gpsimd.dma_start(
                    out=bc[0:64, :],
                    in_=dscr[2 * hp:2 * hp + 1, qs].to_broadcast([64, QB]))
                nc.gpsimd.dma_start(
                    out=bc[64:128, :],
                    in_=dscr[2 * hp + 1:2 * hp + 2, qs].to_broadcast([64, QB]))
                nc.vector.tensor_tensor(
                    out=outuT[hp][:, qs], in0=outuT[hp][:, qs],
                    in1=bc, op=OP.mult)

        def emit_final(t):
            pf = psum_f.tile([128, DIM], F32, name="pf", tag="fin")
            for c in range(NC_):
                nc.tensor.matmul(
                    pf,
                    lhsT=outuT[c][:, t * 128:(t + 1) * 128],
                    rhs=wo[c],
                    start=(c == 0), stop=False)
            nc.tensor.matmul(
                pf,
                lhsT=na8[0:8, t * 128:(t + 1) * 128],
                rhs=W_vt,
                start=False, stop=True)
            fo = fo_p.tile([128, DIM], F32, name="fo", tag="fo")
            nc.vector.tensor_tensor(out=fo, in0=pf, in1=bout_bc, op=OP.add)
            nc.sync.dma_start(out=out_d[t * 128:(t + 1) * 128, :], in_=fo)

        # --- PE issue order ---
        emit_qkT(0)
        emit_dots(0, 0)
        emit_tab_kv()
        emit_qkT(1)
        emit_dots(1, 0)
        emit_qkT(2)
        emit_qkT(3)

        # tab dots: one packed M=8 matmul chain per q block + single exp
        for qb in range(NQB):
            ptab = psum_o.tile([8, QB], F32, name="ptab", tag="o")
            for c in range(NC_):
                nc.tensor.matmul(
                    ptab,
                    lhsT=kpad[:, 8 * c:8 * c + 8],
                    rhs=qT[c][:, qb * QB:(qb + 1) * QB],
                    start=(c == 0), stop=(c == NC_ - 1))
            nc.scalar.activation(
                out=tabexp8[:, qb * QB:(qb + 1) * QB],
                in_=ptab, func=AF.Exp, scale=SCALE)
        # repack exp(tab dots) into the strided staging layout for the
        # denominator add (DMA moves rows to arbitrary partitions)
        for h in range(HEADS):
            r, gc = slot(h)
            nc.gpsimd.dma_start(
                out=tabstr[r:r + 1, gc:gc + NTOK],
                in_=tabexp8[h:h + 1, :])

        # W_vt = v_t @ w_out (K=8-packed blocks against wo chunks)
        psW = psum_f.tile([8, DIM], F32, name="psW", tag="fin")
        for c in range(NC_):
            nc.tensor.matmul(
                psW,
                lhsT=vpad[:, 8 * c:8 * c + 8],
                rhs=wo[c],
                start=(c == 0), stop=(c == NC_ - 1))
        nc.vector.tensor_copy(out=W_vt, in_=psW)

        # v token-major (+ ones interleave)
        for t in range(NT):
            ps = big_tile("psv")
            pv = ps[:, 0:QB]
            for kc in range(NC_):
                nc.tensor.matmul(
                    pv,
                    lhsT=xnT[kc][:, t * 128:(t + 1) * 128],
                    rhs=wq[kc][:, 2 * INNER:3 * INNER],
                    start=(kc == 0), stop=(kc == NC_ - 1))
            vdst = v_sb[t].rearrange("p (h s) -> p h s", s=65)
            nc.vector.tensor_copy(out=vdst[:, :, 0:64],
                                  in_=pv.rearrange("p (h d) -> p h d", d=64))
            nc.vector.tensor_copy(
                out=vdst[:, :, 64:65],
                in_=ones8.rearrange("p (h o) -> p h o", o=1))

        # attention steady state (attn@v interleaved with next dots;
        # normalization per head-group as soon as its denominators land)
        emit_attnv(0, 0); emit_dots(2, 0)
        emit_attnv(1, 0); emit_norm_g(0, 0); emit_dots(3, 0)
        emit_attnv(2, 0); emit_dots(0, 1)
        emit_attnv(3, 0); emit_norm_g(0, 1); emit_dots(1, 1)
        emit_attnv(0, 1); emit_dots(2, 1)
        emit_final(0); emit_final(1)
        emit_attnv(1, 1); emit_norm_g(1, 0); emit_dots(3, 1)
        emit_final(2); emit_final(3)
        emit_attnv(2, 1)
        emit_attnv(3, 1)
        emit_norm_g(1, 1)
        for t in range(4, 8):
            emit_final(t)


_CACHED_NC = None


def _to_bf16(a):
    import ml_dtypes
    return np.ascontiguousarray(np.asarray(a, dtype=np.float32)).astype(
        ml_dtypes.bfloat16)


def kernel(**inputs):
    global _CACHED_NC
    img = np.ascontiguousarray(np.asarray(inputs["img"], dtype=np.float32))
    tab = np.ascontiguousarray(np.asarray(inputs["tab"], dtype=np.float32))
    w_qkv = _to_bf16(inputs["w_qkv"])
    w_tab_qkv = np.asarray(inputs["w_tab_qkv"], dtype=np.float32)
    w_tab = _to_bf16(w_tab_qkv[:, INNER:3 * INNER])
    w_out = _to_bf16(inputs["w_out"])
    b_out = np.asarray(inputs["b_out"], dtype=np.float32).reshape(1, DIM)
    ln_w = np.asarray(inputs["ln_w"], dtype=np.float32).reshape(1, DIM)
    ln_b = np.asarray(inputs["ln_b"], dtype=np.float32).reshape(1, DIM)

    if _CACHED_NC is None:
        _CACHED_NC = build_program()
    nc = _CACHED_NC

    in_maps = []
    for b in range(N_CORES):
        in_maps.append({
            "img_s": np.ascontiguousarray(img[b]),
            "tab_s": np.ascontiguousarray(tab[b]),
            "wq_b": w_qkv,
            "wt_b": w_tab,
            "wo_b": w_out,
            "b_out": b_out,
            "ln_w": ln_w,
            "ln_b": ln_b,
        })

    res = bass_utils.run_bass_kernel_spmd(nc, in_maps, core_ids=list(range(N_CORES)))
    out = np.stack([res.results[c]["out_s"] for c in range(N_CORES)], axis=0)
    return out.astype(np.float32)


if __name__ == "__main__":
    d = np.load("/root/problem/ref_data.npz")
    ins = {k: d[k] for k in ("img", "tab", "w_qkv", "w_tab_qkv", "w_out",
                             "b_out", "ln_w", "ln_b")}
    actual = kernel(**ins)
    expected = d["expected"]
    err = np.abs(actual - expected).max()
    rel = err / np.abs(expected).max()
    print("absmax err:", err, "rel:", rel)
